# revision 5
# baseline (speedup 1.0000x reference)
"""Trainium2 Bass kernel for nn_ConceptIntergation (histogram_binning).

Reference computation:
    counts[b,s,n] = sum_k one_hot(concepts[b,s,k], 129)[..., n]  (n < 128; 128 = padding)
    out[b,s,n,d]  = counts[b,s,n] * emb_table[n,d]

Strategy (data-parallel over batch, 8 cores; transposed class-major layout):
  - Each core handles B_LOC=8 batches -> R=1600 (b,s) rows. The kernel is
    HBM-store bound (26.2 MB bf16 per core, ~358 GB/s per-core DMA), so
    everything else is organized to keep the 16 SDMA engines saturated.
  - Device layout puts the CONCEPT CLASS n on the partition axis:
      countsT[n, r] = sum_k (concepts[r,k] == n)
      out_d[n, r]   = emb[n, d] * countsT[n, r]
    With n on partitions, emb[:, d] is a per-partition scalar, so the big
    multiply runs as DVE tensor_scalar in the 4x perf mode (bf16, step-1,
    SBUF) at ~480ns per [128,1600] slice -- 2x faster than the best
    tensor_tensor, and the Scalar engine (activation Copy with per-partition
    scale AP) computes slices concurrently. The histogram is 4 big DVE ops
    total (is_equal against the partition-index column, accumulated over
    K=4) instead of 4 per 128-row block.
  - Host prep: indices are replicated across partitions and laid out k-major
    ([128, K*R] bf16) so the histogram in0 is step-1 contiguous; emb_table
    passes through as [128, 64] f32 (per-partition scalars must be f32).
  - Output shards are [128(n), 64(d) * 1600(r)] bf16, stored in multi-d
    groups (>=0.8 MB per dma_start). Host transposes to [r, n, d] and
    upcasts bf16->f32 with an exact bit shift.
"""

import numpy as np
import ml_dtypes

import concourse.bass as bass
import concourse.mybir as mybir
from concourse import bacc
from concourse.tile import TileContext
from concourse.bass_utils import run_bass_kernel_spmd

B, S, K = 64, 200, 4
N, D = 128, 64
NCORES = 8
B_LOC = B // NCORES             # 8
R = B_LOC * S                   # 1600 (b,s) rows per core
P = 128
RK = K * R                      # 6400
OUTW = D * R                    # 102400

# d-slice groups per engine: (engine, d_start, d_end). DVE owns d 0..47 with
# a small first group so the first store issues early; ScalarE owns d 48..63.
# DVE stores issue on the Sync HWDGE ring, ScalarE stores on the ACT HWDGE
# ring so the two engines' store issue never serializes.
_DVE_GROUPS = [(0, 2), (2, 6), (6, 12), (12, 18), (18, 24), (24, 30), (30, 36), (36, 42), (42, 48)]
_SCE_GROUPS = [(48, 56), (56, 64)]

BF16 = mybir.dt.bfloat16
F32 = mybir.dt.float32

_NC_CACHE = {}


def _build_nc():
    nc = bacc.Bacc()
    idxb = nc.declare_dram_parameter("idxb", [P, RK], BF16, isOutput=False)
    emb = nc.declare_dram_parameter("emb", [P, D], F32, isOutput=False)
    pcol = nc.declare_dram_parameter("pcol", [P, 1], F32, isOutput=False)
    out = nc.declare_dram_parameter("out", [P, OUTW], BF16, isOutput=True)

    with TileContext(nc) as tc:
        with (
            tc.tile_pool(name="const", bufs=1) as cpool,
            tc.tile_pool(name="vout", bufs=5) as vpool,
            tc.tile_pool(name="sout", bufs=2) as spool,
        ):
            pcol_sb = cpool.tile([P, 1], F32)
            nc.sync.dma_start(out=pcol_sb, in_=pcol[:, :])
            # k-major replicated indices; 4 chunk loads so the histogram
            # chain starts as soon as chunk 0 lands.
            idx_sb = cpool.tile([P, RK], BF16)
            for k in range(K):
                nc.sync.dma_start(
                    out=idx_sb[:, k * R : (k + 1) * R],
                    in_=idxb[:, k * R : (k + 1) * R],
                )
            emb_sb = cpool.tile([P, D], F32)
            nc.sync.dma_start(out=emb_sb, in_=emb[:, :])

            # warm the ScalarE activation table during the input DMAs
            warm = cpool.tile([P, 1], F32)
            nc.scalar.copy(out=warm, in_=pcol_sb)

            # countsT[n, r] = sum_k (idx[r,k] == n)
            counts = cpool.tile([P, R], BF16)
            nc.vector.tensor_scalar(
                out=counts,
                in0=idx_sb[:, 0:R],
                scalar1=pcol_sb[:, 0:1],
                scalar2=None,
                op0=mybir.AluOpType.is_equal,
            )
            for k in range(1, K):
                nc.vector.scalar_tensor_tensor(
                    out=counts,
                    in0=idx_sb[:, k * R : (k + 1) * R],
                    scalar=pcol_sb[:, 0:1],
                    in1=counts,
                    op0=mybir.AluOpType.is_equal,
                    op1=mybir.AluOpType.add,
                )

            def emit_group(pool, tag, d0, d1, use_vector):
                gd = d1 - d0
                ob = pool.tile([P, gd * R], BF16, tag=tag)
                for i in range(gd):
                    dd = d0 + i
                    dst = ob[:, i * R : (i + 1) * R]
                    if use_vector:
                        nc.vector.tensor_scalar(
                            out=dst,
                            in0=counts,
                            scalar1=emb_sb[:, dd : dd + 1],
                            scalar2=None,
                            op0=mybir.AluOpType.mult,
                        )
                    else:
                        nc.scalar.mul(out=dst, in_=counts, mul=emb_sb[:, dd : dd + 1])
                eng = nc.sync if use_vector else nc.scalar
                eng.dma_start(out=out[:, d0 * R : d1 * R], in_=ob)

            for d0, d1 in _SCE_GROUPS:
                emit_group(spool, "sob", d0, d1, use_vector=False)
            for d0, d1 in _DVE_GROUPS:
                emit_group(vpool, "vob", d0, d1, use_vector=True)

    nc.finalize()
    return nc


def _get_nc():
    if "nc" not in _NC_CACHE:
        _NC_CACHE["nc"] = _build_nc()
    return _NC_CACHE["nc"]


def _prepare_in_maps(concepts, emb_table):
    concepts = np.asarray(concepts)
    emb = np.ascontiguousarray(np.asarray(emb_table, dtype=np.float32))

    # per-core k-major index shards replicated across the 128 partitions
    conc = concepts.reshape(NCORES, R, K)
    idx_kmaj = np.ascontiguousarray(conc.transpose(0, 2, 1)).reshape(NCORES, 1, RK)
    idx_bcast = np.broadcast_to(
        idx_kmaj.astype(ml_dtypes.bfloat16), (NCORES, P, RK)
    )
    idx_dev = np.ascontiguousarray(idx_bcast)

    pcol = np.arange(P, dtype=np.float32).reshape(P, 1)
    return [
        {"idxb": idx_dev[i], "emb": emb, "pcol": pcol}
        for i in range(NCORES)
    ]


def _run(concepts, emb_table, **spmd_kwargs):
    nc = _get_nc()
    in_maps = _prepare_in_maps(concepts, emb_table)
    res = run_bass_kernel_spmd(nc, in_maps, core_ids=list(range(NCORES)), **spmd_kwargs)
    # shards are [128(n), 64(d)*1600(r)] bf16; -> [r, n, d], upcast exactly
    u16 = np.stack(
        [np.asarray(res.results[i]["out"]).view(np.uint16) for i in range(NCORES)]
    ).reshape(NCORES, N, D, R)
    u16 = u16.transpose(0, 3, 1, 2)  # -> [core, r, n, d]
    f32 = (u16.astype(np.uint32) << 16).view(np.float32)
    out = f32.reshape(B, S, N, D)
    return out, res


def kernel(concepts, emb_table):
    out, _ = _run(concepts, emb_table)
    return out
